# revision 1
# baseline (speedup 1.0000x reference)
"""LinearSelfAttention kernel for TRN2 (8 NeuronCores, batch-parallel).

Computes out = H + (PH @ mask(H^T Q H)) / n per sample, re-associated as
    HtQ = H^T Q            [s, e]
    PHt = (P H)^T          [s, d]
    Ct  = HtQ[:n]^T PHt[:n]  [e, d]   (mask = drop s == n row)
    out = H + (Ct/n)^T H
which is O(n d^2) instead of O(n^2 d).

Sharding: data-parallel over batch, 2 samples per core, P/Q replicated.
Matmuls in bf16 (fp32 PSUM accumulate); the fp32 H is added in the
epilogue on DVE so the dominant H term stays exact.
"""

import sys

sys.path.insert(0, "/opt/trn_rl_repo")

import numpy as np
import ml_dtypes

B, D1, N1 = 16, 257, 2049  # batch, d+1, n+1
N = N1 - 1  # 2048
NCORES = 8
BPC = B // NCORES  # samples per core

# partition chunking of the 257-sized dims: (offset, size)
CH = [(0, 128), (128, 128), (256, 1)]
NT = N // 128  # 16 full s-tiles (s == 2048 row is masked off)
# t chunks for the final matmul free dim
TCH = [(i * 512, min(512, N1 - i * 512)) for i in range((N1 + 511) // 512)]

_cached = {}


def _build():
    import concourse.bass as bass
    import concourse.tile as tile
    from concourse import bacc, mybir
    from contextlib import ExitStack

    f32 = mybir.dt.float32
    bf16 = mybir.dt.bfloat16

    nc = bacc.Bacc("TRN2", target_bir_lowering=False, debug=False, num_devices=NCORES)

    H_d = nc.declare_dram_parameter("H", [BPC, D1, N1], f32, isOutput=False)
    Hb_d = nc.declare_dram_parameter("Hb", [BPC, D1, N1], bf16, isOutput=False)
    QP_d = nc.declare_dram_parameter("QP", [D1, 514], bf16, isOutput=False)
    Y_d = nc.declare_dram_parameter("Y", [BPC, D1, N1], f32, isOutput=True)

    with tile.TileContext(nc) as tc:
        with ExitStack() as ctx:
            const = ctx.enter_context(tc.tile_pool(name="const", bufs=1))
            hfp = ctx.enter_context(tc.tile_pool(name="hfp", bufs=2))
            hbp = ctx.enter_context(tc.tile_pool(name="hbp", bufs=2))
            sq = ctx.enter_context(tc.tile_pool(name="sq", bufs=2))
            ctp = ctx.enter_context(tc.tile_pool(name="ctp", bufs=2))
            yp = ctx.enter_context(tc.tile_pool(name="yp", bufs=6))

            # ---- input DMAs, spread across engine queues so the first
            # s-tile's operands land as early as possible
            qp = []
            for c, (off, sz) in enumerate(CH):
                t = const.tile([128, 514], bf16, tag=f"qp{c}", name=f"qp{c}")
                nc.sync.dma_start(t[:sz, :], QP_d[off : off + sz, :])
                qp.append(t)

            hf = [[None] * 3 for _ in range(BPC)]
            hb = [[None] * 3 for _ in range(BPC)]
            # Priority class 0: sample-0 bf16 H (gates the first matmuls).
            # One tile per queue so it gets the full HBM bandwidth.
            load_eng = {0: nc.sync, 1: nc.scalar, 2: nc.gpsimd}
            for c, (off, sz) in enumerate(CH):
                tb = hbp.tile([128, N1], bf16, tag=f"hb{c}", name=f"hb0_{c}")
                load_eng[c].dma_start(tb[:sz, :], Hb_d[0, off : off + sz, :])
                hb[0][c] = tb
            # Priority class 1+2 (gpsimd, gated behind class 0 by probe
            # copies): sample-1 bf16 H, then the fp32 H for the epilogue.
            probe = const.tile([128, 16], bf16, tag="probe", name="probe")
            nc.gpsimd.tensor_copy(probe[0:1, 0:8], hb[0][0][0:1, 0:8])
            nc.gpsimd.tensor_copy(probe[0:1, 8:16], hb[0][1][0:1, 0:8])
            for c, (off, sz) in enumerate(CH):
                tb = hbp.tile([128, N1], bf16, tag=f"hb{c}", name=f"hb1_{c}")
                nc.gpsimd.dma_start(tb[:sz, :], Hb_d[1, off : off + sz, :])
                hb[1][c] = tb
            for b in range(BPC):
                for c, (off, sz) in enumerate(CH):
                    tf = hfp.tile([128, N1], f32, tag=f"hf{c}", name=f"hf{b}_{c}")
                    nc.gpsimd.dma_start(tf[:sz, :], H_d[b, off : off + sz, :])
                    hf[b][c] = tf

            # ---- PE warmup: dummy matmuls bridge the input-DMA latency and
            # push the HAM clock gate to K=8/8. Results never read.
            wsb = const.tile([128, 128], bf16, tag="wsb", name="wsb")
            nc.vector.memset(wsb[:, :], 0.0)
            with tc.tile_pool(name="wp", bufs=1, space="PSUM") as wp:
                wps = wp.tile([128, 512], f32, tag="wps", name="warm_ps")
                NWARM = 145
                for i in range(NWARM):
                    nc.tensor.matmul(
                        wps[:, 0:128],
                        wsb[:, :],
                        wsb[:, :],
                        start=(i == 0),
                        stop=(i == NWARM - 1),
                    )

            htq = [None] * BPC
            pht = [None] * BPC
            # ---- S1 + S2: HtQ [s,e] and PHt [s,d], 16 s-tiles each ----
            with tc.tile_pool(name="pp12", bufs=3, space="PSUM") as pp12:
                for b in range(BPC):
                    htq[b] = sq.tile([128, NT * 257], bf16, tag="htq", name=f"htq{b}")
                    pht[b] = sq.tile([128, NT * 257], bf16, tag="pht", name=f"pht{b}")
                    for st in range(NT):
                        p_htq = pp12.tile(
                            [128, 257], f32, tag="p_htq", name=f"p_htq{b}_{st}"
                        )
                        p_pht = pp12.tile(
                            [128, 257], f32, tag="p_pht", name=f"p_pht{b}_{st}"
                        )
                        sl = slice(st * 128, (st + 1) * 128)
                        for c, (off, sz) in enumerate(CH):
                            st_flags = dict(start=(c == 0), stop=(c == 2))
                            nc.tensor.matmul(
                                p_htq[:, :],
                                hb[b][c][:sz, sl],
                                qp[c][:sz, 0:257],
                                **st_flags,
                            )
                            nc.tensor.matmul(
                                p_pht[:, :],
                                hb[b][c][:sz, sl],
                                qp[c][:sz, 257:514],
                                **st_flags,
                            )
                        osl = slice(st * 257, (st + 1) * 257)
                        # alternate eviction engines to balance DVE/ACT
                        if st % 2 == 0:
                            nc.vector.tensor_copy(htq[b][:, osl], p_htq[:, :])
                            nc.scalar.copy(pht[b][:, osl], p_pht[:, :])
                        else:
                            nc.scalar.copy(htq[b][:, osl], p_htq[:, :])
                            nc.vector.tensor_copy(pht[b][:, osl], p_pht[:, :])

            # ---- S3: Ct[e,d] = sum_{s<2048} HtQ[s,e] * PHt[s,d], scaled 1/n
            ct = [[None] * 3 for _ in range(BPC)]
            with tc.tile_pool(name="pp3", bufs=3, space="PSUM") as pp3:
                for b in range(BPC):
                    for ec, (eoff, esz) in enumerate(CH):
                        p_ct = pp3.tile([128, 257], f32, tag="p_ct", name=f"p_ct{b}_{ec}")
                        for st in range(NT):
                            base = st * 257
                            nc.tensor.matmul(
                                p_ct[:esz, :],
                                htq[b][:, base + eoff : base + eoff + esz],
                                pht[b][:, base : base + 257],
                                start=(st == 0),
                                stop=(st == NT - 1),
                            )
                        t = ctp.tile([128, 257], bf16, tag=f"ct{ec}", name=f"ct{b}_{ec}")
                        nc.scalar.mul(t[:esz, :], p_ct[:esz, :], 1.0 / N)
                        ct[b][ec] = t

            # ---- S4: Y = H + (Ct/n)^T H ----
            with tc.tile_pool(name="pp4", bufs=4, space="PSUM") as pp4:
                for b in range(BPC):
                    for dc, (doff, dsz) in enumerate(CH):
                        y = yp.tile([128, N1], f32, tag="y", name=f"y{b}_{dc}")
                        for toff, tsz in TCH:
                            p_a = pp4.tile(
                                [128, 512], f32, tag="p_a", name=f"p_a{b}_{dc}_{toff}"
                            )
                            for ec, (eoff, esz) in enumerate(CH):
                                nc.tensor.matmul(
                                    p_a[:dsz, :tsz],
                                    ct[b][ec][:esz, doff : doff + dsz],
                                    hb[b][ec][:esz, toff : toff + tsz],
                                    start=(ec == 0),
                                    stop=(ec == 2),
                                )
                            nc.vector.tensor_add(
                                y[:dsz, toff : toff + tsz],
                                p_a[:dsz, :tsz],
                                hf[b][dc][:dsz, toff : toff + tsz],
                            )
                            # store each chunk as soon as its epilogue add is
                            # done; alternate queues so store issue keeps up
                            st_eng = nc.sync if (toff // 512) % 2 == 0 else nc.scalar
                            st_eng.dma_start(
                                Y_d[b, doff : doff + dsz, toff : toff + tsz],
                                y[:dsz, toff : toff + tsz],
                            )

    nc.compile()
    return nc


def _prep_in_maps(H, P, Q):
    H = np.ascontiguousarray(H, dtype=np.float32)
    Hb = H.astype(ml_dtypes.bfloat16)
    QP = np.ascontiguousarray(
        np.concatenate([Q, P.T], axis=1).astype(ml_dtypes.bfloat16)
    )
    return [
        {
            "H": H[c * BPC : (c + 1) * BPC],
            "Hb": Hb[c * BPC : (c + 1) * BPC],
            "QP": QP,
        }
        for c in range(NCORES)
    ]


def kernel(H, P, Q):
    from concourse.bass_utils import run_bass_kernel_spmd

    if "nc" not in _cached:
        _cached["nc"] = _build()
    nc = _cached["nc"]

    in_maps = _prep_in_maps(H, P, Q)
    res = run_bass_kernel_spmd(nc, in_maps, list(range(NCORES)))
    out = np.concatenate([res.results[c]["Y"] for c in range(NCORES)], axis=0)
    return out.astype(np.float32)



# revision 5
# speedup vs baseline: 1.5646x; 1.5646x over previous
"""LinearSelfAttention kernel for TRN2 (8 NeuronCores, batch-parallel).

out = H + (P @ mask(H^T Q H)) ... reassociated via the Gram matrix:
    G  = H' H'^T                [257, 257]   (H' = first n=2048 columns)
    At = Q^T G (P^T / n)        [257, 257]   (= (P G Q / n)^T)
    outT = H^T + H^T At         [2049, 257]  (computed transposed, t on partitions)

Host prep supplies H in both layouts (native bf16 + transposed bf16 tiles) and
zero-pads every 257-sized contraction to 384 so all matmuls are K=128 (K=1
stationary loads serialize on the PE weight-buffer). Output is stored
transposed in bf16; the host transposes back and casts to fp32.

Sharding: data-parallel over batch, 2 samples per core, P/Q replicated.
"""

import sys

sys.path.insert(0, "/opt/trn_rl_repo")

import numpy as np
import ml_dtypes

B, D1, N1 = 16, 257, 2049  # batch, d+1, n+1
N = N1 - 1  # 2048
NCORES = 8
BPC = B // NCORES  # samples per core
NT = 16  # full 128-row tiles of s (s=2048 handled as 17th partial tile)
TW = 257  # tile width (e dim)
NWARM = 30

_cached = {}


def _build():
    import concourse.bass as bass
    import concourse.tile as tile
    from concourse import bacc, mybir
    from concourse.masks import make_identity
    from contextlib import ExitStack

    f32 = mybir.dt.float32
    bf16 = mybir.dt.bfloat16

    nc = bacc.Bacc("TRN2", target_bir_lowering=False, debug=False, num_devices=NCORES)

    # Htb: H^T tiles, partition-major: [b][p, st*257+e] = H[b, e, st*128+p]
    Htb_d = nc.declare_dram_parameter("Htb", [BPC, 128, 17 * TW], bf16, isOutput=False)
    # Hb: native layout bf16
    Hb_d = nc.declare_dram_parameter("Hb", [BPC, D1, N1], bf16, isOutput=False)
    # CP: [384, 641] = [Q_pad (384 cols) | P^T/n pad (257 cols)]
    CP_d = nc.declare_dram_parameter("CP", [384, 641], bf16, isOutput=False)
    # YT: transposed output, partition-major like Htb
    YT_d = nc.declare_dram_parameter("YT", [BPC, 128, 17 * TW], bf16, isOutput=True)

    with tile.TileContext(nc) as tc:
        with ExitStack() as ctx:
            const = ctx.enter_context(tc.tile_pool(name="const", bufs=1))
            htp = ctx.enter_context(tc.tile_pool(name="htp", bufs=2))
            hbp = ctx.enter_context(tc.tile_pool(name="hbp", bufs=2))
            gcp = ctx.enter_context(tc.tile_pool(name="gcp", bufs=2))
            chp = ctx.enter_context(tc.tile_pool(name="chp", bufs=2))
            otp = ctx.enter_context(tc.tile_pool(name="otp", bufs=2))

            # ---- input DMAs ------------------------------------------------
            cpt = []
            for kc in range(3):
                t = const.tile([128, 641], bf16, tag=f"cp{kc}", name=f"cp{kc}")
                nc.scalar.dma_start(t[:, :], CP_d[kc * 128 : (kc + 1) * 128, :])
                cpt.append(t)

            # Htb: sync queue, first chunks first (G of sample 0 gates start)
            htb = []
            HTCH = [(0, 5), (5, 5), (10, 5), (15, 2)]  # st chunks
            for b in range(BPC):
                t = htp.tile([128, 17 * TW], bf16, tag="htb", name=f"htb{b}")
                for c0, cn in HTCH:
                    nc.sync.dma_start(
                        t[:, c0 * TW : (c0 + cn) * TW],
                        Htb_d[b, :, c0 * TW : (c0 + cn) * TW],
                    )
                htb.append(t)

            # Hb: scalar queue (sample 0), gpsimd queue (sample 1)
            hb = []
            for b in range(BPC):
                eng = nc.scalar if b == 0 else nc.gpsimd
                t0 = hbp.tile([128, N1], bf16, tag="hb0", name=f"hb0_{b}")
                t1 = hbp.tile([128, N1], bf16, tag="hb1", name=f"hb1_{b}")
                t2 = hbp.tile([128, N1], bf16, tag="hb2", name=f"hb2_{b}")
                eng.dma_start(t0[:, :], Hb_d[b, 0:128, :])
                eng.dma_start(t1[:, :], Hb_d[b, 128:256, :])
                nc.vector.memset(t2[:, :], 0.0)
                eng.dma_start(t2[0:1, :], Hb_d[b, 256:257, :])
                hb.append((t0, t1, t2))

            ident = const.tile([128, 128], bf16, tag="ident", name="ident")
            make_identity(nc, ident[:, :])

            # zero-padded tiles for K=128 uniformity
            gc2 = []
            m1c2 = []
            for b in range(BPC):
                g = gcp.tile([128, TW], bf16, tag="gc2", name=f"gc2_{b}")
                m = chp.tile([128, TW], bf16, tag="m1c2", name=f"m1c2_{b}")
                nc.vector.memset(g[:, :], 0.0)
                nc.vector.memset(m[:, :], 0.0)
                gc2.append(g)
                m1c2.append(m)

            # ---- PE warmup: bridge input-DMA latency, open the HAM gate ----
            wsb = const.tile([128, 128], bf16, tag="wsb", name="wsb")
            nc.vector.memset(wsb[:, :], 0.0)
            with tc.tile_pool(name="wp", bufs=1, space="PSUM") as wp:
                wps = wp.tile([128, 512], f32, tag="wps", name="warm_ps")
                for i in range(NWARM):
                    nc.tensor.matmul(
                        wps[:, 0:128],
                        wsb[:, :],
                        wsb[:, :],
                        start=(i == 0),
                        stop=(i == NWARM - 1),
                    )

            # ---- G = H' H'^T (symmetric: upper blocks only) ---------------
            # psum layout per sample: [0:257]=G[0:128,:], [257:386]=G[128:256,128:257],
            # [386:387]=G[256,256]
            gc0 = [None] * BPC
            gc1 = [None] * BPC
            with tc.tile_pool(name="gp", bufs=2, space="PSUM") as gp, tc.tile_pool(
                name="tp", bufs=3, space="PSUM"
            ) as tp:
                gps = [None] * BPC
                for b in range(BPC):
                    gps[b] = gp.tile([128, 387], f32, tag="gps", name=f"gps{b}")
                    for st in range(NT):
                        o = st * TW
                        nc.tensor.matmul(
                            gps[b][:, 0:257],
                            htb[b][:, o : o + 128],
                            htb[b][:, o : o + 257],
                            start=(st == 0),
                            stop=(st == NT - 1),
                        )
                        nc.tensor.matmul(
                            gps[b][:, 257:386],
                            htb[b][:, o + 128 : o + 256],
                            htb[b][:, o + 128 : o + 257],
                            start=False,
                            stop=(st == NT - 1),
                            skip_group_check=True,
                        )
                        nc.tensor.matmul(
                            gps[b][0:1, 386:387],
                            htb[b][:, o + 256 : o + 257],
                            htb[b][:, o + 256 : o + 257],
                            start=False,
                            stop=(st == NT - 1),
                            skip_group_check=True,
                        )
                    # ---- reconstruct full G in SBUF (bf16) --------------
                    g0 = gcp.tile([128, TW], bf16, tag="gc0", name=f"gc0_{b}")
                    g1 = gcp.tile([128, TW], bf16, tag="gc1", name=f"gc1_{b}")
                    nc.vector.tensor_copy(g0[:, :], gps[b][:, 0:257])
                    nc.scalar.copy(g1[:, 128:257], gps[b][:, 257:386])
                    # G[128:256,0:128] = G[0:128,128:256]^T
                    pt0 = tp.tile([128, 128], bf16, tag="pt", name=f"pt0_{b}")
                    nc.tensor.transpose(pt0[:, :], g0[:, 128:256], ident[:, :])
                    nc.scalar.copy(g1[:, 0:128], pt0[:, :])
                    # G[256,0:128] = G[0:128,256]^T ; G[256,128:256] = G[128:256,256]^T
                    pt1 = tp.tile([128, 128], bf16, tag="pt", name=f"pt1_{b}")
                    nc.tensor.transpose(pt1[0:1, 0:128], g0[:, 256:257], ident[:, :])
                    nc.scalar.copy(gc2[b][0:1, 0:128], pt1[0:1, 0:128])
                    pt2 = tp.tile([128, 128], bf16, tag="pt", name=f"pt2_{b}")
                    nc.tensor.transpose(pt2[0:1, 0:128], g1[:, 256:257], ident[:, :])
                    nc.scalar.copy(gc2[b][0:1, 128:256], pt2[0:1, 0:128])
                    nc.scalar.copy(gc2[b][0:1, 256:257], gps[b][0:1, 386:387])
                    gc0[b] = g0
                    gc1[b] = g1

            # ---- chain: M1 = G @ (P^T/n);  At = Q^T @ M1 -------------------
            MSL = [(0, 128), (128, 128), (256, 1)]  # output row chunks of M1
            at = [[None] * 3 for _ in range(BPC)]
            with tc.tile_pool(name="cp2", bufs=3, space="PSUM") as cpp:
                m1c = [[None] * 3 for _ in range(BPC)]
                for b in range(BPC):
                    gcs = [gc0[b], gc1[b], gc2[b]]
                    for mc, (mo, msz) in enumerate(MSL):
                        p = cpp.tile([128, TW], f32, tag="m1p", name=f"m1p{b}_{mc}")
                        for kc in range(3):
                            nc.tensor.matmul(
                                p[:msz, :],
                                gcs[kc][:, mo : mo + msz],
                                cpt[kc][:, 384:641],
                                start=(kc == 0),
                                stop=(kc == 2),
                            )
                        if mc == 0:
                            t = chp.tile([128, TW], bf16, tag="m1c0", name=f"m1c0_{b}")
                            nc.vector.tensor_copy(t[:, :], p[:, :])
                            m1c[b][0] = t
                        elif mc == 1:
                            t = chp.tile([128, TW], bf16, tag="m1c1", name=f"m1c1_{b}")
                            nc.scalar.copy(t[:, :], p[:, :])
                            m1c[b][1] = t
                        else:
                            nc.scalar.copy(m1c2[b][0:1, :], p[0:1, :])
                            m1c[b][2] = m1c2[b]
                for b in range(BPC):
                    for mc in range(3):
                        qo = mc * 128
                        p = cpp.tile([128, TW], f32, tag="atp", name=f"atp{b}_{mc}")
                        for kc in range(3):
                            nc.tensor.matmul(
                                p[:, :],
                                cpt[kc][:, qo : qo + 128],
                                m1c[b][kc][:, :],
                                start=(kc == 0),
                                stop=(kc == 2),
                            )
                        t = chp.tile([128, TW], bf16, tag=f"at{mc}", name=f"at{b}_{mc}")
                        if mc == 0:
                            nc.vector.tensor_copy(t[:, :], p[:, :])
                        else:
                            nc.scalar.copy(t[:, :], p[:, :])
                        at[b][mc] = t

            # ---- final: outT = Ht + Ht @ At, stored transposed ------------
            with tc.tile_pool(name="fp", bufs=4, space="PSUM") as fpp:
                for b in range(BPC):
                    ot = otp.tile([128, 17 * TW], bf16, tag="ot", name=f"ot{b}")
                    hbs = hb[b]
                    for st in range(17):
                        if st < NT:
                            tsl = slice(st * 128, (st + 1) * 128)
                            rows = 128
                        else:
                            tsl = slice(2048, 2049)
                            rows = 1
                        p = fpp.tile([128, TW], f32, tag="pa", name=f"pa{b}_{st}")
                        for ec in range(3):
                            nc.tensor.matmul(
                                p[:rows, :],
                                hbs[ec][:, tsl],
                                at[b][ec][:, :],
                                start=(ec == 0),
                                stop=(ec == 2),
                            )
                        o = st * TW
                        nc.vector.tensor_add(
                            ot[:rows, o : o + TW],
                            p[:rows, :],
                            htb[b][:rows, o : o + TW],
                        )
                        # store in 4/4/4/5-tile chunks as epilogues complete
                        if st in (3, 7, 11, 16):
                            c0 = {3: 0, 7: 4, 11: 8, 16: 12}[st] * TW
                            c1 = (st + 1) * TW
                            eng = nc.sync if (st % 8 == 3) else nc.scalar
                            eng.dma_start(
                                YT_d[b, :, c0:c1],
                                ot[:, c0:c1],
                            )

    nc.compile()
    return nc


def _prep_in_maps(H, P, Q):
    H = np.ascontiguousarray(H, dtype=np.float32)
    Hb = H.astype(ml_dtypes.bfloat16)
    # Htb[b]: [128, 17*257]; [p, st*257+e] = H[b, e, st*128+p]
    HtT = np.zeros((B, 17 * 128, D1), dtype=ml_dtypes.bfloat16)
    HtT[:, :N1, :] = np.ascontiguousarray(H.transpose(0, 2, 1)).astype(
        ml_dtypes.bfloat16
    )
    Htb = np.ascontiguousarray(
        HtT.reshape(B, 17, 128, D1).transpose(0, 2, 1, 3)
    ).reshape(B, 128, 17 * D1)
    CP = np.zeros((384, 641), dtype=np.float32)
    CP[:D1, :D1] = Q
    CP[:D1, 384 : 384 + D1] = P.T / N
    CP = CP.astype(ml_dtypes.bfloat16)
    return [
        {
            "Htb": Htb[c * BPC : (c + 1) * BPC],
            "Hb": Hb[c * BPC : (c + 1) * BPC],
            "CP": CP,
        }
        for c in range(NCORES)
    ]


def _post(res):
    out = np.empty((B, D1, N1), dtype=np.float32)
    for c in range(NCORES):
        yt = np.asarray(res.results[c]["YT"]).astype(np.float32)
        # [BPC, 128, 17*257] -> [BPC, 17, 128, 257] -> [BPC, 2176, 257]
        yt = yt.reshape(BPC, 128, 17, D1).transpose(0, 2, 1, 3).reshape(BPC, -1, D1)
        out[c * BPC : (c + 1) * BPC] = yt[:, :N1, :].transpose(0, 2, 1)
    return out


def kernel(H, P, Q):
    from concourse.bass_utils import run_bass_kernel_spmd

    if "nc" not in _cached:
        _cached["nc"] = _build()
    nc = _cached["nc"]

    in_maps = _prep_in_maps(H, P, Q)
    res = run_bass_kernel_spmd(nc, in_maps, list(range(NCORES)))
    return _post(res)
